# revision 5
# baseline (speedup 1.0000x reference)
"""nn_Decoder_76948634075330 — Bass kernel for 8 TRN2 NeuronCores.

kernel(**inputs) takes the FULL inputs (a,b int32 [1024,3]; Wn,Wm f32
[128,64]; bn,bm f32 [64]) and returns (P [1024,1024] f32, gathered [4096]
int32), matching reference.py.

Distribution: data-parallel over the M (query) axis — each core computes a
128-row shard of Nmat and the matching 128 columns of Mmat, all-gathers the
Mmat column blocks on-device, and computes its 128 rows of P = Nmat @ Mmat.
Host does only shard/prep, final assembly, and the top-k index selection
(the top-k ordering of near-ties is numerically chaotic at fp32 — see
work/numerics_exp.py — so it is done exactly once on the assembled P).

Per-core pipeline, per 8-m batch:
  - DVE builds Morton tokens for 8 m values packed on partitions
    (p = 16*j + o), in fp32-exact integer arithmetic (mod/is_ge).
  - Per m: PE broadcasts the 16 token rows to the 128-feature layout
    (selector matmul), DVE is_equal -> one-hot features (bf16), PE matmul
    with [Wn|Wm]/64 in bf16 hi/lo (fp32-accurate), ACT relu(+bias),
    GPSIMD partition-reduce -> Nmat row / Mmat column.
"""

import numpy as np
import ml_dtypes

M = 1024
N = 1024
DIM = 3
OFF = 16
CLS = 8
FEAT = 128
K = 64
MAXPTS = 4096
NCORES = 8
MSH = M // NCORES  # 128 m rows per core

BF16 = ml_dtypes.bfloat16

LAST_RESULT = None  # BassKernelResults of the most recent device run
_CACHE = {}


def _host_prep(a, b, Wn, bn, Wm, bm):
    Wcat = np.concatenate([Wn, Wm], axis=1).astype(np.float32) / np.float32(K)
    Whi = Wcat.astype(BF16)
    Wlo = (Wcat - Whi.astype(np.float32)).astype(BF16)
    biasK = (np.concatenate([bn, bm]) / np.float32(K)).astype(np.float32).reshape(FEAT, 1)

    A128 = np.broadcast_to(a.T.reshape(1, DIM * N), (128, DIM * N)).copy().astype(np.float32)
    o_of_p = np.arange(128) % OFF
    MASKW = np.broadcast_to((np.int32(1) << o_of_p).astype(np.int32).reshape(128, 1),
                            (128, N)).copy()
    P2D = np.stack([np.float32(2.0) ** (d - o_of_p.astype(np.float32)) for d in range(DIM)],
                   axis=1).astype(np.float32)  # [128, 3]
    C8 = (np.arange(128) % CLS).astype(np.float32).reshape(128, 1)
    E128s = np.zeros((128, 8 * 128), dtype=BF16)
    for j in range(8):
        for p in range(128):
            E128s[16 * j + p // CLS, 128 * j + p] = 1
    I128 = np.eye(128, dtype=np.float32)

    nbat = MSH // 8
    in_maps = []
    for core in range(NCORES):
        m0 = core * MSH
        B128 = np.zeros((128, 3 * nbat), dtype=np.float32)
        for bat in range(nbat):
            for d in range(DIM):
                B128[:, 3 * bat + d] = b[m0 + 8 * bat + np.arange(128) // OFF, d]
        in_maps.append(dict(
            A128=A128, B128=B128, MASKW=MASKW, P2D=P2D,
            C8=C8, E128s=E128s, Whi=Whi, Wlo=Wlo, biasK=biasK, I128=I128,
        ))
    return in_maps


def _build_body(nc, tc, ins, outs, msh):
    from concourse import mybir
    from contextlib import ExitStack
    f32 = mybir.dt.float32
    bf16 = mybir.dt.bfloat16
    Alu = mybir.AluOpType
    AF = mybir.ActivationFunctionType

    nbat = msh // 8
    H = 512

    es = ExitStack()
    const = es.enter_context(tc.tile_pool(name="const", bufs=1))
    tokscr = es.enter_context(tc.tile_pool(name="tokscr", bufs=1))
    tokp = es.enter_context(tc.tile_pool(name="tokp", bufs=2))
    featp = es.enter_context(tc.tile_pool(name="featp", bufs=3))
    Rbp = es.enter_context(tc.tile_pool(name="Rbp", bufs=2))
    tmpp = es.enter_context(tc.tile_pool(name="tmpp", bufs=1))
    s2p = es.enter_context(tc.tile_pool(name="s2p", bufs=1))
    acc = es.enter_context(tc.tile_pool(name="acc", bufs=1))
    psA = es.enter_context(tc.tile_pool(name="psA", bufs=2, space="PSUM"))
    psB = es.enter_context(tc.tile_pool(name="psB", bufs=2, space="PSUM"))
    psT = es.enter_context(tc.tile_pool(name="psT", bufs=2, space="PSUM"))
    psP = es.enter_context(tc.tile_pool(name="psP", bufs=2, space="PSUM"))

    def load_const(name, shape, dt):
        t = const.tile(shape, dt, tag=name)
        nc.sync.dma_start(t[:], ins[name])
        return t

    i32 = mybir.dt.int32
    A128 = load_const("A128", [128, DIM * N], f32)
    B128 = load_const("B128", [128, 3 * nbat], f32)
    MASKW = load_const("MASKW", [128, N], i32)
    P2D = load_const("P2D", [128, 3], f32)
    C8 = load_const("C8", [128, 1], f32)
    E128s = load_const("E128s", [128, 8 * 128], bf16)
    Whi = load_const("Whi", [FEAT, FEAT], bf16)
    Wlo = load_const("Wlo", [FEAT, FEAT], bf16)
    biasK = load_const("biasK", [FEAT, 1], f32)
    I128 = load_const("I128", [128, 128], f32)

    NmatSh = acc.tile([msh, N], f32, tag="NmatSh")
    McolSh = acc.tile([msh, N], f32, tag="McolSh")
    # DRAM staging for the partition-reduce outputs: row (2*bat+h) holds the
    # [1, 8*H] strip for that (batch, half) — SBUF free->partition scatter in
    # a single DMA mis-lowers on HW, so stage through DRAM instead.
    nm_dram = nc.dram_tensor("nm_stage", [2 * nbat, 8 * H], f32, kind="Internal")
    mc_dram = nc.dram_tensor("mc_stage", [2 * nbat, 8 * H], f32, kind="Internal")

    for bat in range(nbat):
        # token build, 8 m packed on partitions p = 16*j + o, fp32/int32 exact:
        #   s_d = int32(a_d + b_d); w_d = s_d & 2^o; tok = sum_d w_d * 2^(d-o)
        ws = []
        for d in range(DIM):
            s_i = tokscr.tile([128, N], i32, tag=f"s{d}")
            nc.vector.tensor_scalar(
                out=s_i[:], in0=A128[:, d * N:(d + 1) * N],
                scalar1=B128[:, 3 * bat + d:3 * bat + d + 1], scalar2=None,
                op0=Alu.add)
            w_i = tokscr.tile([128, N], i32, tag=f"w{d}")
            nc.vector.tensor_tensor(out=w_i[:], in0=s_i[:], in1=MASKW[:],
                                    op=Alu.bitwise_and)
            ws.append(w_i)
        bv0 = tokscr.tile([128, N], f32, tag="bv0")
        t01 = tokscr.tile([128, N], f32, tag="t01")
        tokf = tokp.tile([128, N], bf16, tag="tokf")
        nc.vector.tensor_scalar(out=bv0[:], in0=ws[0][:], scalar1=P2D[:, 0:1],
                                scalar2=None, op0=Alu.mult)
        nc.vector.scalar_tensor_tensor(out=t01[:], in0=ws[1][:], scalar=P2D[:, 1:2],
                                       in1=bv0[:], op0=Alu.mult, op1=Alu.add)
        nc.vector.scalar_tensor_tensor(out=tokf[:], in0=ws[2][:], scalar=P2D[:, 2:3],
                                       in1=t01[:], op0=Alu.mult, op1=Alu.add)

        for h in range(2):
            Rb = Rbp.tile([128, 8 * H], f32, tag="Rb")
            for j in range(8):
                tokbc = psA.tile([128, H], f32, tag="tokbc")
                nc.tensor.matmul(tokbc[:], lhsT=E128s[:, 128 * j:128 * (j + 1)],
                                 rhs=tokf[:, h * H:(h + 1) * H],
                                 start=True, stop=True)
                feat = featp.tile([128, H], bf16, tag="feat")
                nc.vector.tensor_scalar(out=feat[:], in0=tokbc[:], scalar1=C8[:],
                                        scalar2=None, op0=Alu.is_equal)
                act = psB.tile([128, H], f32, tag="act")
                nc.tensor.matmul(act[:], lhsT=Whi[:], rhs=feat[:], start=True, stop=False)
                nc.tensor.matmul(act[:], lhsT=Wlo[:], rhs=feat[:], start=False, stop=True)
                nc.scalar.activation(out=Rb[:, j * H:(j + 1) * H], in_=act[:],
                                     func=AF.Relu, bias=biasK[:], scale=1.0)
            tmpN = tmpp.tile([1, 8 * H], f32, tag="tmpN")
            tmpM = tmpp.tile([1, 8 * H], f32, tag="tmpM")
            nc.gpsimd.tensor_reduce(out=tmpN[:], in_=Rb[0:K, :],
                                    axis=mybir.AxisListType.C, op=Alu.add)
            nc.gpsimd.tensor_reduce(out=tmpM[:], in_=Rb[K:2 * K, :],
                                    axis=mybir.AxisListType.C, op=Alu.add)
            nc.sync.dma_start(nm_dram.ap()[2 * bat + h:2 * bat + h + 1, :], tmpN[:])
            nc.sync.dma_start(mc_dram.ap()[2 * bat + h:2 * bat + h + 1, :], tmpM[:])

    # gather the staged strips into [m, n] SBUF layout:
    # NmatSh[8*bat+j, 512*h+q] = nm_dram[2*bat+h, 512*j+q]
    for t_sb, t_dr in ((NmatSh, nm_dram), (McolSh, mc_dram)):
        for bat in range(nbat):
            for h in range(2):
                nc.sync.dma_start(
                    t_sb[8 * bat:8 * bat + 8, h * H:(h + 1) * H],
                    t_dr.ap()[2 * bat + h].rearrange("(j q) -> j q", j=8))

    # ---- stage 2 ----
    ag_in = nc.dram_tensor("ag_in", [N, msh], f32, kind="Internal")
    ag_out = nc.dram_tensor("ag_out", [NCORES * N, msh], f32, kind="Internal",
                            addr_space="Shared")
    NmatT = []
    for c in range(8):
        pt = psT.tile([128, msh], f32, tag="pt")
        nc.tensor.transpose(pt[:], NmatSh[:, c * 128:(c + 1) * 128], I128[0:msh, 0:msh])
        nt = s2p.tile([128, msh], f32, tag=f"NmatT{c}")
        nc.scalar.copy(nt[:], pt[:])
        NmatT.append(nt)
        pt2 = psT.tile([128, msh], f32, tag="pt")
        nc.tensor.transpose(pt2[:], McolSh[:, c * 128:(c + 1) * 128], I128[0:msh, 0:msh])
        mt = s2p.tile([128, msh], f32, tag="mt")
        nc.scalar.copy(mt[:], pt2[:])
        nc.sync.dma_start(ag_in.ap()[c * 128:(c + 1) * 128, :], mt[:])

    nc.gpsimd.collective_compute(
        "AllGather", mybir.AluOpType.bypass,
        replica_groups=[list(range(NCORES))],
        ins=[ag_in.ap()], outs=[ag_out.ap()])

    agv = ag_out.ap().rearrange("(j c n) q -> c n j q", j=NCORES, c=8, n=128)
    Pout_s = s2p.tile([msh, NCORES * msh], f32, tag="Pout_s")
    Mfull = []
    for c in range(8):
        mf = s2p.tile([128, NCORES * msh], f32, tag=f"Mfull{c}")
        nc.sync.dma_start(mf[:].rearrange("n (j q) -> n j q", j=NCORES), agv[c])
        Mfull.append(mf)
    PW = min(512, NCORES * msh)
    for hh in range((NCORES * msh) // PW):
        pp = psP.tile([msh, PW], f32, tag="pp")
        for c in range(8):
            nc.tensor.matmul(pp[:], lhsT=NmatT[c][:, 0:msh],
                             rhs=Mfull[c][:, hh * PW:(hh + 1) * PW],
                             start=(c == 0), stop=(c == 7))
        nc.scalar.copy(Pout_s[:, hh * PW:(hh + 1) * PW], pp[:])

    nc.sync.dma_start(outs["P_out"], Pout_s[:])
    nc.sync.dma_start(outs["Nm_out"], NmatSh[:])
    nc.sync.dma_start(outs["Mc_out"], McolSh[:])
    es.close()


def _build_program(msh=MSH):
    from concourse import bacc, tile, mybir
    f32 = mybir.dt.float32
    bf16 = mybir.dt.bfloat16
    nc = bacc.Bacc("TRN2", target_bir_lowering=False, debug=False,
                   num_devices=NCORES)
    nbat = msh // 8
    ins = {}
    for name, shape, dt in [
        ("A128", [128, DIM * N], f32), ("B128", [128, 3 * nbat], f32),
        ("MASKW", [128, N], mybir.dt.int32), ("P2D", [128, 3], f32),
        ("C8", [128, 1], f32), ("E128s", [128, 8 * 128], bf16),
        ("Whi", [FEAT, FEAT], bf16), ("Wlo", [FEAT, FEAT], bf16),
        ("biasK", [FEAT, 1], f32), ("I128", [128, 128], f32),
    ]:
        ins[name] = nc.dram_tensor(name, shape, dt, kind="ExternalInput").ap()
    outs = {}
    for name, shape in [("P_out", [msh, NCORES * msh]), ("Nm_out", [msh, N]),
                        ("Mc_out", [msh, N])]:
        outs[name] = nc.dram_tensor(name, shape, f32, kind="ExternalOutput").ap()
    with tile.TileContext(nc) as tc:
        _build_body(nc, tc, ins, outs, msh)
    nc.compile()
    return nc


def _finalize_topk(a, P):
    """top_k over P.flatten() + gather, matching jax.lax.top_k tie-breaking."""
    flat = P.reshape(-1)
    idx = np.argpartition(-flat, MAXPTS)[:MAXPTS]
    order = np.lexsort((idx, -flat[idx]))
    idx = idx[order]
    vals = flat[idx]
    g = a.reshape(-1)[idx % a.size]
    g = np.where(vals > 0, g, 0).astype(np.int32)
    return g


def kernel(a, b, Wn, bn, Wm, bm):
    global LAST_RESULT
    a = np.asarray(a, dtype=np.int32)
    b = np.asarray(b, dtype=np.int32)
    Wn = np.asarray(Wn, dtype=np.float32)
    bn = np.asarray(bn, dtype=np.float32)
    Wm = np.asarray(Wm, dtype=np.float32)
    bm = np.asarray(bm, dtype=np.float32)

    from concourse import bass_utils

    if "nc" not in _CACHE:
        _CACHE["nc"] = _build_program()
    nc = _CACHE["nc"]

    in_maps = _host_prep(a, b, Wn, bn, Wm, bm)
    res = bass_utils.run_bass_kernel_spmd(nc, in_maps, core_ids=list(range(NCORES)))
    LAST_RESULT = res

    P = np.empty((M, M), np.float32)
    for core in range(NCORES):
        P[core * MSH:(core + 1) * MSH, :] = res.results[core]["P_out"]
    gathered = _finalize_topk(a, P)
    return P, gathered


# revision 8
# speedup vs baseline: 60.2246x; 60.2246x over previous
"""nn_Decoder_76948634075330 — Bass kernel for 8 TRN2 NeuronCores.

kernel(**inputs) takes the FULL inputs (a,b int32 [1024,3]; Wn,Wm f32
[128,64]; bn,bm f32 [64]) and returns (P [1024,1024] f32, gathered [4096]
int32), matching reference.py.

Distribution: data-parallel over the M (query) axis — each core computes a
128-row shard of Nmat and the matching 128 columns of Mmat, all-gathers the
Mmat column blocks on-device, and computes its 128 rows of P = Nmat @ Mmat.
Host does only shard/prep, final assembly, and the top-k index selection
(the top-k ordering of near-ties is numerically chaotic at fp32 — see
work/numerics_exp.py — so it is done exactly once on the assembled P).

Per-core pipeline, per 8-m batch:
  - DVE builds Morton tokens for 8 m values packed on partitions
    (p = 16*j + o), in fp32-exact integer arithmetic (mod/is_ge).
  - Per m: PE broadcasts the 16 token rows to the 128-feature layout
    (selector matmul), DVE is_equal -> one-hot features (bf16), PE matmul
    with [Wn|Wm]/64 in bf16 hi/lo (fp32-accurate), ACT relu(+bias),
    GPSIMD partition-reduce -> Nmat row / Mmat column.
"""

import numpy as np
import ml_dtypes

M = 1024
N = 1024
DIM = 3
OFF = 16
CLS = 8
FEAT = 128
K = 64
MAXPTS = 4096
NCORES = 8
MSH = M // NCORES  # 128 m rows per core

BF16 = ml_dtypes.bfloat16

LAST_RESULT = None  # BassKernelResults of the most recent device run
_CACHE = {}


def _host_prep(a, b, Wn, bn, Wm, bm):
    Wcat = np.concatenate([Wn, Wm], axis=1).astype(np.float32) / np.float32(K)
    Whi = Wcat.astype(BF16)
    Wlo = (Wcat - Whi.astype(np.float32)).astype(BF16)
    biasK = (np.concatenate([bn, bm]) / np.float32(K)).astype(np.float32).reshape(FEAT, 1)

    A128 = np.broadcast_to(a.T.reshape(1, DIM * N), (128, DIM * N)).copy().astype(np.float32)
    o_of_p = np.arange(128) % OFF
    MASKW = np.broadcast_to((np.int32(1) << o_of_p).astype(np.int32).reshape(128, 1),
                            (128, N)).copy()
    P2D = np.stack([np.float32(2.0) ** (d - o_of_p.astype(np.float32)) for d in range(DIM)],
                   axis=1).astype(np.float32)  # [128, 3]
    C8 = (np.arange(128) % CLS).astype(np.float32).reshape(128, 1)
    E128s = np.zeros((128, 8 * 128), dtype=BF16)
    for j in range(8):
        for p in range(128):
            E128s[16 * j + p // CLS, 128 * j + p] = 1
    I128 = np.eye(128, dtype=np.float32)
    maskP = np.zeros((128, 32), dtype=np.float32)
    for slot in range(4):
        maskP[:K, 8 * slot + slot] = 1.0
        maskP[K:, 8 * slot + 4 + slot] = 1.0

    nbat = MSH // 8
    in_maps = []
    for core in range(NCORES):
        m0 = core * MSH
        B128 = np.zeros((128, 3 * nbat), dtype=np.float32)
        for bat in range(nbat):
            for d in range(DIM):
                B128[:, 3 * bat + d] = b[m0 + 8 * bat + np.arange(128) // OFF, d]
        in_maps.append(dict(
            A128=A128, B128=B128, MASKW=MASKW, P2D=P2D,
            C8=C8, E128s=E128s, Whi=Whi, Wlo=Wlo, biasK=biasK, I128=I128,
            maskP=maskP,
        ))
    return in_maps


def _build_body(nc, tc, ins, outs, msh):
    from concourse import mybir
    from contextlib import ExitStack
    f32 = mybir.dt.float32
    bf16 = mybir.dt.bfloat16
    Alu = mybir.AluOpType
    AF = mybir.ActivationFunctionType

    nbat = msh // 8
    H = 512

    es = ExitStack()
    const = es.enter_context(tc.tile_pool(name="const", bufs=1))
    tokscr = es.enter_context(tc.tile_pool(name="tokscr", bufs=1))
    tokp = es.enter_context(tc.tile_pool(name="tokp", bufs=2))
    featp = es.enter_context(tc.tile_pool(name="featp", bufs=3))
    Rbp = es.enter_context(tc.tile_pool(name="Rbp", bufs=2))
    s2p = es.enter_context(tc.tile_pool(name="s2p", bufs=1))
    acc = es.enter_context(tc.tile_pool(name="acc", bufs=1))
    psA = es.enter_context(tc.tile_pool(name="psA", bufs=2, space="PSUM"))
    psB = es.enter_context(tc.tile_pool(name="psB", bufs=2, space="PSUM"))
    psQ = es.enter_context(tc.tile_pool(name="psQ", bufs=2, space="PSUM"))

    def load_const(name, shape, dt):
        t = const.tile(shape, dt, tag=name)
        nc.sync.dma_start(t[:], ins[name])
        return t

    i32 = mybir.dt.int32
    A128 = load_const("A128", [128, DIM * N], f32)
    B128 = load_const("B128", [128, 3 * nbat], f32)
    MASKW = load_const("MASKW", [128, N], i32)
    P2D = load_const("P2D", [128, 3], f32)
    C8 = load_const("C8", [128, 1], f32)
    E128s = load_const("E128s", [128, 8 * 128], bf16)
    Whi = load_const("Whi", [FEAT, FEAT], bf16)
    Wlo = load_const("Wlo", [FEAT, FEAT], bf16)
    biasK = load_const("biasK", [FEAT, 1], f32)
    I128 = load_const("I128", [128, 128], f32)
    maskP = load_const("maskP", [128, 32], f32)

    NmatSh = acc.tile([msh, N], f32, tag="NmatSh")
    McolSh = acc.tile([msh, N], f32, tag="McolSh")
    # DRAM staging for the partition-reduce outputs: row (2*bat+h) holds the
    # [1, 8*H] strip for that (batch, half) — SBUF free->partition scatter in
    # a single DMA mis-lowers on HW, so stage through DRAM instead.
    nm_dram = nc.dram_tensor("nm_stage", [4 * nbat, 4 * H], f32, kind="Internal")
    mc_dram = nc.dram_tensor("mc_stage", [4 * nbat, 4 * H], f32, kind="Internal")

    for bat in range(nbat):
        # token build, 8 m packed on partitions p = 16*j + o, fp32/int32 exact:
        #   s_d = int32(a_d + b_d); w_d = s_d & 2^o; tok = sum_d w_d * 2^(d-o)
        ws = []
        for d in range(DIM):
            s_i = tokscr.tile([128, N], i32, tag=f"s{d}")
            nc.vector.tensor_scalar(
                out=s_i[:], in0=A128[:, d * N:(d + 1) * N],
                scalar1=B128[:, 3 * bat + d:3 * bat + d + 1], scalar2=None,
                op0=Alu.add)
            w_i = tokscr.tile([128, N], i32, tag=f"w{d}")
            nc.vector.tensor_tensor(out=w_i[:], in0=s_i[:], in1=MASKW[:],
                                    op=Alu.bitwise_and)
            ws.append(w_i)
        bv0 = tokscr.tile([128, N], f32, tag="bv0")
        t01 = tokscr.tile([128, N], f32, tag="t01")
        tokf = tokp.tile([128, N], bf16, tag="tokf")
        nc.vector.tensor_scalar(out=bv0[:], in0=ws[0][:], scalar1=P2D[:, 0:1],
                                scalar2=None, op0=Alu.mult)
        nc.vector.scalar_tensor_tensor(out=t01[:], in0=ws[1][:], scalar=P2D[:, 1:2],
                                       in1=bv0[:], op0=Alu.mult, op1=Alu.add)
        nc.vector.scalar_tensor_tensor(out=tokf[:], in0=ws[2][:], scalar=P2D[:, 2:3],
                                       in1=t01[:], op0=Alu.mult, op1=Alu.add)

        for h in range(2):
            for g in range(2):
                nmq = psQ.tile([8, H], f32, tag="nmq")
                for s in range(4):
                    j = 4 * g + s
                    tokbc = psA.tile([128, H], f32, tag="tokbc")
                    nc.tensor.matmul(tokbc[:], lhsT=E128s[:, 128 * j:128 * (j + 1)],
                                     rhs=tokf[:, h * H:(h + 1) * H],
                                     start=True, stop=True)
                    feat = featp.tile([128, H], bf16, tag="feat")
                    nc.vector.tensor_scalar(out=feat[:], in0=tokbc[:], scalar1=C8[:],
                                            scalar2=None, op0=Alu.is_equal)
                    act = psB.tile([128, H], f32, tag="act")
                    nc.tensor.matmul(act[:], lhsT=Whi[:], rhs=feat[:], start=True, stop=False)
                    nc.tensor.matmul(act[:], lhsT=Wlo[:], rhs=feat[:], start=False, stop=True)
                    R = Rbp.tile([128, H], f32, tag="R")
                    nc.scalar.activation(out=R[:], in_=act[:],
                                         func=AF.Relu, bias=biasK[:], scale=1.0)
                    # rows 0-3 of nmq: Nmat for slots 0-3; rows 4-7: Mmat cols
                    nc.tensor.matmul(nmq[:], lhsT=maskP[:, 8 * s:8 * s + 8], rhs=R[:],
                                     start=(s == 0), stop=(s == 3))
                nms = Rbp.tile([8, H], f32, tag="nms")
                nc.scalar.copy(nms[:], nmq[:])
                r = (2 * bat + h) * 2 + g
                nc.sync.dma_start(nm_dram.ap()[r].rearrange("(j q) -> j q", j=4),
                                  nms[0:4, :])
                nc.sync.dma_start(mc_dram.ap()[r].rearrange("(j q) -> j q", j=4),
                                  nms[4:8, :])

    # gather the staged strips into [m, n] SBUF layout:
    # NmatSh[8*bat+j, 512*h+q] = nm_dram[2*bat+h, 512*j+q]
    for t_sb, t_dr in ((NmatSh, nm_dram), (McolSh, mc_dram)):
        for bat in range(nbat):
            for h in range(2):
                for g in range(2):
                    r = (2 * bat + h) * 2 + g
                    nc.sync.dma_start(
                        t_sb[8 * bat + 4 * g:8 * bat + 4 * g + 4, h * H:(h + 1) * H],
                        t_dr.ap()[r].rearrange("(j q) -> j q", j=4))

    # ---- stage 2 ----
    ag_in = nc.dram_tensor("ag_in", [N, msh], f32, kind="Internal")
    ag_out = nc.dram_tensor("ag_out", [NCORES * N, msh], f32, kind="Internal",
                            addr_space="Shared")
    NmatT = []
    for c in range(8):
        pt = psA.tile([128, msh], f32, tag="tokbc")
        nc.tensor.transpose(pt[:], NmatSh[:, c * 128:(c + 1) * 128], I128[0:msh, 0:msh])
        nt = s2p.tile([128, msh], f32, tag=f"NmatT{c}")
        nc.scalar.copy(nt[:], pt[:])
        NmatT.append(nt)
        pt2 = psA.tile([128, msh], f32, tag="tokbc")
        nc.tensor.transpose(pt2[:], McolSh[:, c * 128:(c + 1) * 128], I128[0:msh, 0:msh])
        mt = s2p.tile([128, msh], f32, tag="mt")
        nc.scalar.copy(mt[:], pt2[:])
        nc.sync.dma_start(ag_in.ap()[c * 128:(c + 1) * 128, :], mt[:])

    nc.gpsimd.collective_compute(
        "AllGather", mybir.AluOpType.bypass,
        replica_groups=[list(range(NCORES))],
        ins=[ag_in.ap()], outs=[ag_out.ap()])

    agv = ag_out.ap().rearrange("(j c n) q -> c n j q", j=NCORES, c=8, n=128)
    Pout_s = s2p.tile([msh, NCORES * msh], f32, tag="Pout_s")
    Mfull = []
    for c in range(8):
        mf = s2p.tile([128, NCORES * msh], f32, tag=f"Mfull{c}")
        nc.sync.dma_start(mf[:].rearrange("n (j q) -> n j q", j=NCORES), agv[c])
        Mfull.append(mf)
    PW = min(512, NCORES * msh)
    for hh in range((NCORES * msh) // PW):
        pp = psB.tile([msh, PW], f32, tag="act")
        for c in range(8):
            nc.tensor.matmul(pp[:], lhsT=NmatT[c][:, 0:msh],
                             rhs=Mfull[c][:, hh * PW:(hh + 1) * PW],
                             start=(c == 0), stop=(c == 7))
        nc.scalar.copy(Pout_s[:, hh * PW:(hh + 1) * PW], pp[:])

    nc.sync.dma_start(outs["P_out"], Pout_s[:])
    nc.sync.dma_start(outs["Nm_out"], NmatSh[:])
    nc.sync.dma_start(outs["Mc_out"], McolSh[:])
    es.close()


def _build_program(msh=MSH):
    from concourse import bacc, tile, mybir
    f32 = mybir.dt.float32
    bf16 = mybir.dt.bfloat16
    nc = bacc.Bacc("TRN2", target_bir_lowering=False, debug=False,
                   num_devices=NCORES)
    nbat = msh // 8
    ins = {}
    for name, shape, dt in [
        ("A128", [128, DIM * N], f32), ("B128", [128, 3 * nbat], f32),
        ("MASKW", [128, N], mybir.dt.int32), ("P2D", [128, 3], f32),
        ("C8", [128, 1], f32), ("E128s", [128, 8 * 128], bf16),
        ("Whi", [FEAT, FEAT], bf16), ("Wlo", [FEAT, FEAT], bf16),
        ("biasK", [FEAT, 1], f32), ("I128", [128, 128], f32),
        ("maskP", [128, 32], f32),
    ]:
        ins[name] = nc.dram_tensor(name, shape, dt, kind="ExternalInput").ap()
    outs = {}
    for name, shape in [("P_out", [msh, NCORES * msh]), ("Nm_out", [msh, N]),
                        ("Mc_out", [msh, N])]:
        outs[name] = nc.dram_tensor(name, shape, f32, kind="ExternalOutput").ap()
    with tile.TileContext(nc) as tc:
        _build_body(nc, tc, ins, outs, msh)
    nc.compile()
    return nc


def _finalize_topk(a, P):
    """top_k over P.flatten() + gather, matching jax.lax.top_k tie-breaking."""
    flat = P.reshape(-1)
    idx = np.argpartition(-flat, MAXPTS)[:MAXPTS]
    order = np.lexsort((idx, -flat[idx]))
    idx = idx[order]
    vals = flat[idx]
    g = a.reshape(-1)[idx % a.size]
    g = np.where(vals > 0, g, 0).astype(np.int32)
    return g


def kernel(a, b, Wn, bn, Wm, bm):
    global LAST_RESULT
    a = np.asarray(a, dtype=np.int32)
    b = np.asarray(b, dtype=np.int32)
    Wn = np.asarray(Wn, dtype=np.float32)
    bn = np.asarray(bn, dtype=np.float32)
    Wm = np.asarray(Wm, dtype=np.float32)
    bm = np.asarray(bm, dtype=np.float32)

    from concourse import bass_utils

    if "nc" not in _CACHE:
        _CACHE["nc"] = _build_program()
    nc = _CACHE["nc"]

    in_maps = _host_prep(a, b, Wn, bn, Wm, bm)
    res = bass_utils.run_bass_kernel_spmd(nc, in_maps, core_ids=list(range(NCORES)))
    LAST_RESULT = res

    P = np.empty((M, M), np.float32)
    for core in range(NCORES):
        P[core * MSH:(core + 1) * MSH, :] = res.results[core]["P_out"]
    gathered = _finalize_topk(a, P)
    return P, gathered
